# revision 1
# baseline (speedup 1.0000x reference)
"""Trainium2 kernel for nn_HashCodeAwareLogits.

Strategy: the 512MB bucket table is row-sharded across the 8 cores
(8192 rows each). Each (b, d, hash) instance is routed (on host) to the
core owning its bucket row. The core indirect-DMA-gathers the 8KB rows,
multiplies by the (importance-weight-scaled) t vector broadcast over the
32 n-ary slots, and reduces over the embedding dim, producing a [32]
logit partial per instance. Host reassembles: out[b,d] = sum over the 2
hash instances.
"""

import math

import ml_dtypes
import numpy as np

import concourse.bass as bass
import concourse.mybir as mybir
from concourse import bacc
from concourse.bass_utils import run_bass_kernel_spmd
from concourse.tile import TileContext

PRIME = (1 << 31) - 1
N_DIGITS = 16
N_ARY = 32
EMB = 64
NUM_EMB = 100000
NUM_BUCKETS = 65536
NUM_HASHES = 2
N_CORES = 8
ROWS_PER_CORE = NUM_BUCKETS // N_CORES  # 8192
P = 128

_rng = np.random.RandomState(42)
SEQ_A = _rng.randint(1, PRIME, size=(N_DIGITS,)).astype(np.int64)
HASH_A = _rng.randint(1, PRIME, size=(NUM_HASHES,)).astype(np.int64)
HASH_B = _rng.randint(0, PRIME, size=(NUM_HASHES,)).astype(np.int64)

TRACE = False
LAST_RESULT = None


def _ensure_ntff_hook():
    """Bridge the axon NTFF profile hook into antenv.axon_hooks (which this
    image lacks) so run_bass_kernel_spmd(trace=True) can capture profiles."""
    import sys
    import types

    if "antenv.axon_hooks" in sys.modules:
        return
    try:
        sys.path.insert(0, "/root/.axon_site/trn_agent_boot")
        import trn_boot  # type: ignore

        hook = trn_boot._ntff_profile_via_ctypes("/opt/axon/libaxon_pjrt.so")
    except Exception:
        hook = None
    mod = types.ModuleType("antenv.axon_hooks")
    mod._hook = hook
    mod.get_axon_ntff_profile_hook = lambda: mod._hook
    mod.set_axon_ntff_profile_hook = lambda h: setattr(mod, "_hook", h)
    sys.modules["antenv.axon_hooks"] = mod

_PROGRAM_CACHE = {}


def _prefix_ids(seq):
    # seq: [B, D] int64, 0 = padding
    h = np.cumsum(SEQ_A[None, :] * (seq % PRIME), axis=-1) % PRIME
    lengths = (seq != 0).sum(axis=-1, keepdims=True)
    pos = np.arange(seq.shape[-1], dtype=np.int64)[None, :]
    idx = np.minimum(pos, np.maximum(lengths - 1, 0))
    return np.take_along_axis(h, idx, axis=-1)  # [B, D]


def _build_program(ntiles):
    nc = bacc.Bacc()
    table = nc.declare_dram_parameter(
        "table", [ROWS_PER_CORE, N_ARY * EMB], mybir.dt.bfloat16, isOutput=False
    )
    idx_d = nc.declare_dram_parameter("idx", [P, ntiles], mybir.dt.int32, isOutput=False)
    tv_d = nc.declare_dram_parameter(
        "tvec", [P, ntiles * EMB], mybir.dt.bfloat16, isOutput=False
    )
    red_d = nc.declare_dram_parameter(
        "red", [ntiles * P, N_ARY], mybir.dt.bfloat16, isOutput=True
    )
    dbg_d = nc.declare_dram_parameter("dbg", [P, 1], mybir.dt.float32, isOutput=True)

    with TileContext(nc) as tc:
        with (
            tc.tile_pool(name="misc", bufs=1) as misc,
            tc.tile_pool(name="gath", bufs=4) as gpool,
            tc.tile_pool(name="prod", bufs=3) as ppool,
            tc.tile_pool(name="small", bufs=4) as spool,
        ):
            idx_sb = misc.tile([P, ntiles], mybir.dt.int32)
            nc.sync.dma_start(out=idx_sb[:, :], in_=idx_d[:, :])
            tv_sb = misc.tile([P, ntiles * EMB], mybir.dt.bfloat16)
            nc.sync.dma_start(out=tv_sb[:, :], in_=tv_d[:, :])

            # sacrificial first touch of tv_sb: carries the load-sem wait so
            # the per-tile TTs below only ever wait on their gather sem.
            dumt = misc.tile([P, 1], mybir.dt.float32)
            nc.vector.tensor_copy(out=dumt[:, :], in_=tv_sb[:, 0:1])
            nc.sync.dma_start(out=dbg_d[:, :], in_=dumt[:, :])

            for t in range(ntiles):
                g = gpool.tile([P, N_ARY * EMB], mybir.dt.bfloat16, tag="g")
                nc.gpsimd.indirect_dma_start(
                    out=g[:, :],
                    out_offset=None,
                    in_=table[:, :],
                    in_offset=bass.IndirectOffsetOnAxis(ap=idx_sb[:, t : t + 1], axis=0),
                )
                prod = ppool.tile([P, N_ARY * EMB], mybir.dt.bfloat16, tag="prod")
                g3 = g[:, :].rearrange("p (a e) -> p a e", e=EMB)
                t3 = (
                    tv_sb[:, t * EMB : (t + 1) * EMB]
                    .rearrange("p (a e) -> p a e", a=1)
                    .to_broadcast([P, N_ARY, EMB])
                )
                nc.vector.tensor_tensor(
                    out=prod[:, :].rearrange("p (a e) -> p a e", e=EMB),
                    in0=g3,
                    in1=t3,
                    op=mybir.AluOpType.mult,
                )
                # binary-tree reduction over e via bf16 TT adds (2x mode),
                # since TensorReduce only runs at 1 elem/cycle.
                cur = prod
                width = EMB
                while width > 1:
                    half = width // 2
                    nxt = spool.tile([P, N_ARY * half], mybir.dt.bfloat16, tag=f"s{half}")
                    cur3 = cur[:, :].rearrange("p (a e) -> p a e", e=width)
                    with nc.allow_low_precision("bf16 tree within rel-err budget"):
                        nc.vector.tensor_tensor(
                            out=nxt[:, :].rearrange("p (a e) -> p a e", e=half),
                            in0=cur3[:, :, 0:half],
                            in1=cur3[:, :, half:width],
                            op=mybir.AluOpType.add,
                        )
                    cur = nxt
                    width = half
                nc.sync.dma_start(out=red_d[t * P : (t + 1) * P, :], in_=cur[:, :])
    nc.finalize()
    return nc


def kernel(input_sequence, t_representation, importance_weights, bucket_table):
    global LAST_RESULT
    input_sequence = np.asarray(input_sequence, dtype=np.int64)
    t_representation = np.asarray(t_representation, dtype=np.float32)
    importance_weights = np.asarray(importance_weights, dtype=np.float32)
    bucket_table = np.asarray(bucket_table, dtype=np.float32)

    B, D = input_sequence.shape
    npos = B * D

    ids = _prefix_ids(input_sequence)  # [B, D]
    ids_f = ids.reshape(-1)  # [npos]
    w_all = importance_weights[ids_f % NUM_EMB]  # [npos, 2]
    t_flat = t_representation.reshape(npos, EMB)

    # per-instance (pos, h) arrays
    pos_arr = np.tile(np.arange(npos, dtype=np.int64), NUM_HASHES)
    h_arr = np.repeat(np.arange(NUM_HASHES, dtype=np.int64), npos)
    bucket_arr = np.concatenate(
        [((HASH_A[h] * ids_f + HASH_B[h]) % PRIME) % NUM_BUCKETS for h in range(NUM_HASHES)]
    )
    w_arr = np.concatenate([w_all[:, h] for h in range(NUM_HASHES)]).astype(np.float32)

    # sort by bucket => core-major (core = bucket // ROWS_PER_CORE), dedup-friendly
    perm = np.argsort(bucket_arr, kind="stable")
    pos_s, h_s, bucket_s, w_s = (
        pos_arr[perm],
        h_arr[perm],
        bucket_arr[perm],
        w_arr[perm],
    )
    table_bf16 = np.ascontiguousarray(bucket_table.astype(ml_dtypes.bfloat16))
    core_s = (bucket_s // ROWS_PER_CORE).astype(np.int64)
    counts = np.bincount(core_s, minlength=N_CORES)
    starts = np.concatenate([[0], np.cumsum(counts)])

    nmax = int(counts.max())
    ntiles = max(1, math.ceil(nmax / P))
    NMAX = ntiles * P

    key = ntiles
    if key not in _PROGRAM_CACHE:
        _PROGRAM_CACHE[key] = _build_program(ntiles)
    nc = _PROGRAM_CACHE[key]

    in_maps = []
    for c in range(N_CORES):
        s, e = starts[c], starts[c + 1]
        n = e - s
        idx_pad = np.zeros(NMAX, dtype=np.int32)
        idx_pad[:n] = (bucket_s[s:e] - c * ROWS_PER_CORE).astype(np.int32)
        tv_pad = np.zeros((NMAX, EMB), dtype=ml_dtypes.bfloat16)
        tv_pad[:n] = (t_flat[pos_s[s:e]] * w_s[s:e, None]).astype(
            ml_dtypes.bfloat16
        )  # fold w into tvec
        in_maps.append(
            {
                "table": table_bf16[c * ROWS_PER_CORE : (c + 1) * ROWS_PER_CORE],
                "idx": np.ascontiguousarray(idx_pad.reshape(ntiles, P).T),
                "tvec": np.ascontiguousarray(
                    tv_pad.reshape(ntiles, P, EMB).transpose(1, 0, 2).reshape(
                        P, ntiles * EMB
                    )
                ),
            }
        )

    if TRACE:
        _ensure_ntff_hook()
    res = run_bass_kernel_spmd(nc, in_maps, list(range(N_CORES)), trace=TRACE)
    LAST_RESULT = res

    out2 = np.zeros((NUM_HASHES, npos, N_ARY), dtype=np.float32)
    for c in range(N_CORES):
        s, e = starts[c], starts[c + 1]
        n = e - s
        red = np.asarray(res.results[c]["red"])[:n].astype(np.float32)
        out2[h_s[s:e], pos_s[s:e]] = red
    out = out2.sum(axis=0).reshape(B, D, N_ARY)
    return out

